# revision 18
# baseline (speedup 1.0000x reference)
"""TRN2 Bass kernel for nn_Attention_188978561266.

Reference computation (b=4, s=1024, d=1024, 16 heads x 64):
    qkv = x @ Wqkv ; split q,k,v
    q = q / (sqrt(mean(q^2 over ALL elements)) + eps) * scale_q   (global scalar RMS)
    k = k / (sqrt(mean(k^2 over ALL elements)) + eps) * scale_k
    attn = softmax(q @ k^T)  (no 1/sqrt(d_head), no mask)
    out = (attn @ v) @ Wo

Sharding: 8 cores = (batch b in 0..3) x (head-half in 0..1). Each core computes
qkv for its batch restricted to its 8 heads, full attention for those heads,
and a partial output projection in two passes (zparta = head pairs 0,1 of the
local half, zpartb = head pairs 2,3). Host sums the four partials per batch.
The global RMS needs a cross-core AllReduce of two scalars.

Schedule notes:
- The first collective on a fresh execution costs ~60-80us of firmware boot;
  a dummy AllReduce at kernel start absorbs it concurrently with the
  projections. Everything before the real AllReduce result (~95us) is gate
  shadow; warm-keeper matmuls (in the AV psum pool, emitted after the
  pre-gate S matmuls) bridge the PE so the clock gate stays 8/8.
- PSUM: S-logit pool = 2 x [128,3,512] (6 banks); AV/O pool = 1 x [128,2,512]
  (2 banks). S tiles hold 3 (skt, head) chunks -> one exp call each (1536
  elems; ragged 1-chunk tail), and the AV matmuls + O-projection chunks live
  in their own 2-bank ring so they never stall the S/exp pipeline.
- E layout interleaves the two heads (chunk 2*skc + i) so S matmul pairs are
  adjacent 64-row tiles on row groups (0,0)/(64,0) -> concurrent.
- Q/K/S run fp32r (exp amplifies absolute logit error; bf16 there costs ~2%
  output error). V is computed fp32r but stored bf16; AV and O run bf16.
- AV output (z + ones-row denominators) is copied to SBUF immediately so the
  psum slot recycles without waiting for the normalize chain.
- dma_start costs ~0.6us serial issue on the Sync engine -> few, large DMAs.
- The RMS scalar uses exp(0.5*ln(m)) + one Newton step; a dummy Ln up front
  makes walrus load the natural_log_exp table set once for the whole kernel.
"""

import os as _os
import sys

sys.path.insert(0, "/opt/trn_rl_repo")

import numpy as np
from ml_dtypes import bfloat16

import concourse.bacc as bacc
import concourse.mybir as mybir
from concourse import library_config, tile
from concourse.bass_utils import run_bass_kernel_spmd

F32 = mybir.dt.float32
F32R = mybir.dt.float32r
BF16 = mybir.dt.bfloat16
AF = mybir.ActivationFunctionType
ALU = mybir.AluOpType
AX = mybir.AxisListType

P = 128
D = 1024
S = 1024
N_HEAD = 16
DH = 64
NHL = 8          # heads per core
DC = 8           # d contraction chunks of 128
EPS = 1e-6
COUNT = 4 * 1024 * 1024   # elements of the full q (or k) tensor
N_KEEP = int(_os.environ.get("KN_KEEP", "55"))
USE_DUMMY_AR = _os.environ.get("KN_DUMMY", "1") == "1"
N_CORES = int(_os.environ.get("KN_CORES", "8"))
REPLICAS = [list(range(N_CORES))]

_CACHE = {}


def _rne11(x: np.ndarray) -> np.ndarray:
    """Round float32 to 11 explicit mantissa bits (matches HW float32r)."""
    u = np.ascontiguousarray(x, dtype=np.float32).view(np.uint32).astype(np.uint64)
    shift = 12
    bias = ((u >> shift) & 1) + ((1 << (shift - 1)) - 1)
    return (((u + bias) >> shift) << shift).astype(np.uint32).view(np.float32)


def _build():
    nc = bacc.Bacc("TRN2", target_bir_lowering=False, debug=False, num_devices=N_CORES)

    xt = nc.dram_tensor("xt", [P, DC, S], F32R, kind="ExternalInput")
    wqk = nc.dram_tensor("wqk", [P, 8, DC, P], F32R, kind="ExternalInput")
    wv = nc.dram_tensor("wv", [P, DC, NHL * DH], F32R, kind="ExternalInput")
    wo = nc.dram_tensor("wo", [P, 4, D], BF16, kind="ExternalInput")
    qscale = nc.dram_tensor("qscale", [P, 4], F32, kind="ExternalInput")
    zparta = nc.dram_tensor("zparta", [S, D], F32, kind="ExternalOutput")
    zpartb = nc.dram_tensor("zpartb", [S, D], F32, kind="ExternalOutput")

    with tile.TileContext(nc) as tc:
        with (
            tc.tile_pool(name="big", bufs=1) as big,
            tc.tile_pool(name="ep", bufs=2) as ep,
            tc.tile_pool(name="zp", bufs=1) as zp,
            tc.tile_pool(name="scr", bufs=2) as scrp,
            tc.tile_pool(name="ob", bufs=2) as obp,
            tc.tile_pool(name="aux", bufs=1) as auxp,
            tc.tile_pool(name="small", bufs=2) as smallp,
            tc.tile_pool(name="stats", bufs=1) as stp,
            tc.tile_pool(name="s3", bufs=2, space="PSUM") as s3p,
            tc.tile_pool(name="av", bufs=1, space="PSUM") as avp,
            tc.tile_pool(name="dram", bufs=1, space="DRAM") as dramp,
        ):
            # ---- persistent SBUF tensors ----
            xT = big.tile([P, DC, S], F32R, tag="xT")
            wqs = big.tile([P, 8, DC, P], F32R, tag="wqs")
            QT = big.tile([P, 4, S], F32R, tag="QT")
            KT = big.tile([P, 4, S], F32R, tag="KT")
            Vt = big.tile([P, 8, NHL, DH + 1], BF16, tag="Vt")
            zG = big.tile([P, 4, S], BF16, tag="zG")
            Wo_sb = big.tile([P, 4, D], BF16, tag="Wo")
            Wv_sb = big.tile([P, DC, NHL * DH], F32R, tag="Wv")

            qs_sb = stp.tile([P, 4], F32, tag="qs")
            sq_acc = stp.tile([P, 8], F32, tag="sqacc")
            qk2 = stp.tile([P, 2], F32, tag="qk2")
            g_sb = stp.tile([2, 1], F32, tag="gsb")
            gsum = stp.tile([1, 2], F32, tag="gsum")
            sc_a = stp.tile([1, 2], F32, tag="sca")
            sc_b = stp.tile([1, 2], F32, tag="scb")
            sc_c = stp.tile([1, 2], F32, tag="scc")
            pm = stp.tile([1, 1], F32, tag="pm")
            cinv = stp.tile([1, 1], F32, tag="cinv")
            c_bc = stp.tile([P, 1], F32, tag="cbc")
            dln = stp.tile([1, 1], F32, tag="dln")
            ones_col = stp.tile([P, 1], F32, tag="ones_col")
            ones_blk = stp.tile([P, 8, NHL, 1], F32, tag="ones_blk")

            nc.gpsimd.load_library(library_config.attn)
            if USE_DUMMY_AR:
                cc_warm_in = dramp.tile([2, 1], F32, tag="ccwi")
                cc_warm_out = dramp.tile([2, 1], F32, tag="ccwo",
                                         addr_space="Shared")
                nc.gpsimd.collective_compute(
                    "AllReduce",
                    ALU.add,
                    replica_groups=REPLICAS,
                    ins=[cc_warm_in[:]],
                    outs=[cc_warm_out[:]],
                )

            # ---- input DMAs (x and per-ct weights interleaved) ----
            nc.sync.dma_start(wqs[:, 0, :, :], wqk[:, 0, :, :])
            for dc in range(DC):
                nc.sync.dma_start(xT[:, dc, :], xt[:, dc, :])
                if dc >= 1:
                    nc.sync.dma_start(wqs[:, dc, :, :], wqk[:, dc, :, :])
            nc.sync.dma_start(qs_sb[:], qscale[:])
            nc.vector.memset(ones_col[:], 1.0)
            nc.vector.memset(ones_blk[:], 1.0)
            nc.vector.tensor_copy(Vt[:, :, :, DH : DH + 1], ones_blk[:])
            # preload the natural_log_exp ACT table set (covers Square, Ln,
            # Exp for the whole kernel -> no mid-kernel table switches)
            nc.scalar.activation(dln[:], pm[:], AF.Ln)
            nc.sync.dma_start(Wv_sb[:], wv[:])
            nc.sync.dma_start(Wo_sb[:], wo[:])

            # ---- phase A: q,k projections (ct-outer; x streams under dc) ----
            for ct in range(8):
                tA = s3p.tile([P, 3, 512], F32, tag="s3", name=f"tA{ct}")
                for dc in range(DC):
                    for st in range(2):
                        nc.tensor.matmul(
                            tA[:, st, :],
                            lhsT=wqs[:, ct, dc, :],
                            rhs=xT[:, dc, st * 512 : (st + 1) * 512],
                            start=(dc == 0),
                            stop=(dc == DC - 1),
                        )
                view = tA[:, 0:2, :]
                scr = scrp.tile([P, 2, 512], F32, tag="scr", name=f"sq{ct}")
                nc.scalar.activation(
                    scr[:], view, AF.Square, accum_out=sq_acc[:, ct : ct + 1]
                )
                if ct < 4:
                    nc.vector.tensor_scalar(
                        QT[:, ct, :],
                        view.rearrange("p a b -> p (a b)"),
                        qs_sb[:, ct : ct + 1],
                        None,
                        ALU.mult,
                    )
                else:
                    nc.vector.tensor_copy(
                        KT[:, ct - 4, :], view.rearrange("p a b -> p (a b)")
                    )

            # ---- global RMS: local reduce -> AllReduce ----
            nc.vector.reduce_sum(qk2[:, 0:1], sq_acc[:, 0:4], axis=AX.X)
            nc.vector.reduce_sum(qk2[:, 1:2], sq_acc[:, 4:8], axis=AX.X)
            g_ps = avp.tile([P, 2, 512], F32, tag="av", name="g_ps")
            nc.tensor.matmul(
                g_ps[0:2, 0, 0:1], lhsT=qk2[:], rhs=ones_col[:], start=True, stop=True
            )
            nc.vector.tensor_copy(g_sb[:], g_ps[0:2, 0, 0:1])
            cc_in = dramp.tile([2, 1], F32, tag="ccin")
            cc_out = dramp.tile([2, 1], F32, tag="ccout", addr_space="Shared")
            nc.sync.dma_start(cc_in[:], g_sb[:])
            nc.gpsimd.collective_compute(
                "AllReduce",
                ALU.add,
                replica_groups=REPLICAS,
                ins=[cc_in[:]],
                outs=[cc_out[:]],
            )
            nc.sync.dma_start(gsum[:], cc_out[:].rearrange("a b -> b a"))

            # ---- V projection (fp32r, stored bf16), in the gate shadow ----
            for k in range(3):
                sms = range(3 * k, min(3 * k + 3, 8))
                tV = s3p.tile([P, 3, 512], F32, tag="s3", name=f"tV{k}")
                for j, sm in enumerate(sms):
                    for dc in range(DC):
                        nc.tensor.matmul(
                            tV[:, j, :],
                            lhsT=xT[:, dc, sm * P : (sm + 1) * P],
                            rhs=Wv_sb[:, dc, :],
                            start=(dc == 0),
                            stop=(dc == DC - 1),
                        )
                n = len(sms)
                nc.vector.tensor_copy(
                    Vt[:, 3 * k : 3 * k + n, :, 0:DH],
                    tV[:, 0:n, :].rearrange("p a (h d) -> p a h d", h=NHL),
                )

            # ---- RMS scalar chain: sqrt(m) = exp(0.5 ln m), one Newton step ----
            nc.vector.tensor_scalar_mul(sc_a[:], gsum[:], 1.0 / COUNT)   # m
            nc.scalar.activation(sc_c[:], sc_a[:], AF.Ln)                # ln m
            nc.scalar.activation(sc_b[:], sc_c[:], AF.Exp, scale=0.5)    # r0
            nc.vector.reciprocal(sc_c[:], sc_b[:])                       # 1/r0
            nc.vector.tensor_mul(sc_c[:], sc_a[:], sc_c[:])              # m/r0
            nc.vector.tensor_add(sc_b[:], sc_b[:], sc_c[:])              # r0 + m/r0
            nc.vector.tensor_scalar(sc_b[:], sc_b[:], 0.5, EPS, ALU.mult, ALU.add)
            nc.vector.tensor_mul(pm[:], sc_b[:, 0:1], sc_b[:, 1:2])
            nc.vector.reciprocal(cinv[:], pm[:])
            nc.gpsimd.partition_broadcast(c_bc[:], cinv[:])

            def o_chunk(sm, gg0, out_dram):
                tO = avp.tile([P, 2, 512], F32, tag="av", name=f"tO_{gg0}_{sm}")
                for nt in range(2):
                    for gg in (gg0, gg0 + 1):
                        nc.tensor.matmul(
                            tO[:, nt, :],
                            lhsT=zG[:, gg, sm * P : (sm + 1) * P],
                            rhs=Wo_sb[:, gg, nt * 512 : (nt + 1) * 512],
                            start=(gg == gg0),
                            stop=(gg == gg0 + 1),
                        )
                ob = obp.tile([P, 2, 512], F32, tag="ob", name=f"ob_{gg0}_{sm}")
                nc.vector.tensor_copy(ob[:], tO[:])
                nc.sync.dma_start(
                    out_dram[sm * P : (sm + 1) * P, :],
                    ob[:].rearrange("p a b -> p (a b)"),
                )

            def wave_fillers(g, t, E_mix, o_specs):
                """Emission closures finishing wave (g,t): AV quartets, the
                normalize chain, and O-projection chunks. Interleaved between
                the NEXT wave's S tiles so the PE stays uniformly busy."""
                tsl = slice(t * 512, (t + 1) * 512)
                state = {}

                def av_quartet(i, lo):
                    def emit():
                        if (i, lo) == (0, 0):
                            state["tAV"] = avp.tile([P, 2, 512], F32, tag="av",
                                                    name=f"tAV_{g}_{t}")
                        tAV = state["tAV"]
                        l = 2 * g + i
                        for skc in range(lo, lo + 4):
                            nc.tensor.matmul(
                                tAV[0 : DH + 1, i, :],
                                lhsT=Vt[:, skc, l, :],
                                rhs=E_mix[:, 2 * skc + i, :],
                                start=(skc == 0),
                                stop=(skc == 7),
                            )
                    return emit

                def av_done():
                    # raw z + denominators to SBUF so the psum slot recycles
                    tAV = state["tAV"]
                    zAV = zp.tile([DH, 2, 512], F32, tag="zav", name=f"zAV_{g}_{t}")
                    zden = auxp.tile([1, 2, 512], F32, tag="zden",
                                     name=f"zd_{g}_{t}")
                    nc.vector.tensor_copy(zAV[:], tAV[0:DH, 0:2, :])
                    nc.vector.tensor_copy(zden[:], tAV[DH : DH + 1, 0:2, :])
                    rs_r = auxp.tile([1, 2, 512], F32, tag="rs", name=f"rr_{g}_{t}")
                    nc.vector.reciprocal_approx_fast(rs_r[:], zden[:])
                    for i in range(2):
                        bc_sb = smallp.tile([DH, 512], F32, tag="bcs",
                                            name=f"bc_{g}_{t}_{i}")
                        nc.gpsimd.partition_broadcast(bc_sb[:], rs_r[:, i, :])
                        if i == 0:
                            nc.vector.tensor_mul(
                                zG[0:DH, g, tsl], zAV[:, 0, :], bc_sb[:]
                            )
                        else:
                            ztmp = auxp.tile([DH, 512], BF16, tag="ztmp",
                                             name=f"zt_{g}_{t}")
                            nc.vector.tensor_mul(ztmp[:], zAV[:, 1, :], bc_sb[:])
                            nc.sync.dma_start(zG[DH:P, g, tsl], ztmp[:])

                def o_emit(spec):
                    def emit():
                        gg0, sm, dram = spec
                        o_chunk(sm, gg0, dram)
                    return emit

                o_fill = [o_emit(s) for s in o_specs]
                o_fill += [lambda: None] * (2 - len(o_fill))
                fillers = [
                    av_quartet(0, 0),
                    o_fill[0],
                    av_quartet(0, 4),
                    o_fill[1],
                    av_quartet(1, 0),
                    lambda: (av_quartet(1, 4)(), av_done()),
                ]
                return fillers

            def s_tile(g, t, k):
                """S-logit psum tile k (chunks 3k..3k+2) and its exp."""
                tsl = slice(t * 512, (t + 1) * 512)
                cs = range(3 * k, min(3 * k + 3, 16))
                tS = s3p.tile([P, 3, 512], F32, tag="s3", name=f"tS_{g}_{t}_{k}")
                for c in cs:
                    skt, i = c // 2, c % 2
                    hp = i * DH
                    nc.tensor.matmul(
                        tS[:, c - 3 * k, :],
                        lhsT=KT[hp : hp + DH, g, skt * P : (skt + 1) * P],
                        rhs=QT[hp : hp + DH, g, tsl],
                        start=True,
                        stop=True,
                    )
                return tS, cs

            def s_exp(E_mix, tS, cs, k):
                n = len(cs)
                nc.scalar.activation(
                    E_mix[:, 3 * k : 3 * k + n, :], tS[:, 0:n, :], AF.Exp,
                    scale=c_bc[:, 0:1]
                )

            # ---- attention: per (head-pair g, q-half t) wave ----
            # 16 S chunks per wave, chunk c = 2*skt + i; psum tile k = c//3;
            # one exp per tile into E_mix[:, 3k:3k+3, :]. Wave w's AV matmuls,
            # normalize chain, and O chunks are emitted as fillers between
            # wave w+1's S tiles so the PE stays uniformly busy and ACT runs
            # exps back to back. t-outer wave order lets 12 of 16 O chunks
            # run inline; only the (head pairs 2,3) x (sm 4-7) chunks remain
            # for the tail.
            waves = [(g, t) for t in range(2) for g in range(4)]
            # O-chunk queue: (gg0, sm, out_dram), ready once the needed waves'
            # z is finalized (tracked implicitly by emission position).
            o_queue = (
                [(0, sm, zparta) for sm in range(4)]
                + [(2, sm, zpartb) for sm in range(4)]
                + [(0, sm, zparta) for sm in range(4, 8)]
                + [(2, sm, zpartb) for sm in range(4, 8)]
            )
            o_pos = 0
            fillers = None
            for w, (g, t) in enumerate(waves):
                E_mix = ep.tile([P, 16, 512], BF16, tag="E", name=f"E_{g}_{t}")
                for k in range(6):
                    tS, cs = s_tile(g, t, k)
                    if w == 0 and k == 1:
                        # Warm keepers bridge the PE to the AllReduce result
                        # (the exps below gate on c_bc). They live in the AV
                        # pool so they never block the S/exp ring.
                        wk = avp.tile([P, 2, 512], F32, tag="av", name="wk")
                        for r in range(N_KEEP):
                            nc.tensor.matmul(
                                wk[:, r % 2, :],
                                lhsT=KT[:, 0, 0:P],
                                rhs=KT[:, 0, 0:512],
                                start=True,
                                stop=True,
                            )
                    if fillers is not None:
                        fillers[k]()
                    s_exp(E_mix, tS, cs, k)
                n_o = 2 if w >= 2 else 0
                specs = o_queue[o_pos : o_pos + n_o]
                o_pos += n_o
                fillers = wave_fillers(g, t, E_mix, specs)
            # tail: finish the last wave, then the remaining O chunks through
            # the now-free 2-slot S ring
            for f in fillers:
                f()
            for gg0, sm, dram in o_queue[o_pos:]:
                tO = s3p.tile([P, 3, 512], F32, tag="s3", name=f"tO2_{sm}")
                for nt in range(2):
                    for gg in (gg0, gg0 + 1):
                        nc.tensor.matmul(
                            tO[:, nt, :],
                            lhsT=zG[:, gg, sm * P : (sm + 1) * P],
                            rhs=Wo_sb[:, gg, nt * 512 : (nt + 1) * 512],
                            start=(gg == gg0),
                            stop=(gg == gg0 + 1),
                        )
                ob = obp.tile([P, 2, 512], F32, tag="ob", name=f"ob2_{gg0}_{sm}")
                nc.vector.tensor_copy(ob[:], tO[:, 0:2, :])
                nc.sync.dma_start(
                    dram[sm * P : (sm + 1) * P, :],
                    ob[:].rearrange("p a b -> p (a b)"),
                )

    nc.compile()
    return nc


def _get_nc():
    if "nc" not in _CACHE:
        _CACHE["nc"] = _build()
    return _CACHE["nc"]


def _prep_core_inputs(x, Wqkv, Wo, scale_q, scale_k):
    """Host-side shard + layout prep. Returns list of 8 in_maps."""
    x = np.asarray(x, dtype=np.float32)
    Wqkv = np.asarray(Wqkv, dtype=np.float32)
    Wo = np.asarray(Wo, dtype=np.float32)
    scale_q = np.asarray(scale_q, dtype=np.float32)
    scale_k = np.asarray(scale_k, dtype=np.float32)

    # combined per-d_head scale folded into Q (applied after raw sum-sq)
    qs_vec = np.tile(scale_q * scale_k, NHL)               # [512]
    qs_dev = np.ascontiguousarray(qs_vec.reshape(4, P).T)  # [128,4]

    xt_all = []
    for b in range(4):
        xTb = x[b].T                                       # [d, s]
        lay = xTb.reshape(DC, P, S).transpose(1, 0, 2)     # [128, 8, 1024]
        xt_all.append(np.ascontiguousarray(_rne11(lay)))

    in_maps = []
    for c in range(8):
        b = c // 2
        hh = (c % 2) * NHL
        cols = slice(hh * DH, (hh + NHL) * DH)
        wq_c = Wqkv[:, 0 * D:1 * D][:, cols]               # [1024, 512]
        wk_c = Wqkv[:, 1 * D:2 * D][:, cols]
        wv_c = Wqkv[:, 2 * D:3 * D][:, cols]
        wqk_c = _rne11(np.concatenate([wq_c, wk_c], axis=1))  # [1024, 1024]
        # [p, ct, dc, n]: one DMA per ct covers the full contraction
        wqk_dev = np.ascontiguousarray(
            wqk_c.reshape(DC, P, 8, P).transpose(1, 2, 0, 3)
        )
        wv_dev = np.ascontiguousarray(
            _rne11(wv_c).reshape(DC, P, NHL * DH).transpose(1, 0, 2)
        )
        # Wo rows for local heads, arranged [128, 4, 1024]:
        # chunk g partition p = head (2g + p//64), row p%64
        wo_loc = Wo[(hh * DH):(hh + NHL) * DH, :]          # [512, 1024]
        wo_dev = np.empty((P, 4, D), dtype=bfloat16)
        for g in range(4):
            wo_dev[0:DH, g, :] = wo_loc[2 * g * DH:(2 * g + 1) * DH, :].astype(bfloat16)
            wo_dev[DH:P, g, :] = wo_loc[(2 * g + 1) * DH:(2 * g + 2) * DH, :].astype(bfloat16)
        in_maps.append(
            {
                "xt": xt_all[b],
                "wqk": wqk_dev,
                "wv": wv_dev,
                "wo": np.ascontiguousarray(wo_dev),
                "qscale": qs_dev,
            }
        )
    return in_maps


def run(x, Wqkv, Wo, scale_q, scale_k, trace=False):
    nc = _get_nc()
    in_maps = _prep_core_inputs(x, Wqkv, Wo, scale_q, scale_k)
    res = run_bass_kernel_spmd(
        nc, in_maps[:N_CORES], core_ids=list(range(N_CORES)), trace=trace
    )
    out = np.empty((4, S, D), dtype=np.float32)
    for b in range(4):
        if N_CORES == 8:
            out[b] = (
                res.results[2 * b]["zparta"]
                + res.results[2 * b]["zpartb"]
                + res.results[2 * b + 1]["zparta"]
                + res.results[2 * b + 1]["zpartb"]
            )
    return out, res


def kernel(x, Wqkv, Wo, scale_q, scale_k):
    out, _ = run(x, Wqkv, Wo, scale_q, scale_k, trace=False)
    return out


# revision 19
# speedup vs baseline: 1.0207x; 1.0207x over previous
"""TRN2 Bass kernel for nn_Attention_188978561266.

Reference computation (b=4, s=1024, d=1024, 16 heads x 64):
    qkv = x @ Wqkv ; split q,k,v
    q = q / (sqrt(mean(q^2 over ALL elements)) + eps) * scale_q   (global scalar RMS)
    k = k / (sqrt(mean(k^2 over ALL elements)) + eps) * scale_k
    attn = softmax(q @ k^T)  (no 1/sqrt(d_head), no mask)
    out = (attn @ v) @ Wo

Sharding: 8 cores = (batch b in 0..3) x (head-half in 0..1). Each core computes
qkv for its batch restricted to its 8 heads, full attention for those heads,
and a partial output projection in two passes (zparta = head pairs 0,1 of the
local half, zpartb = head pairs 2,3). Host sums the four partials per batch.
The global RMS needs a cross-core AllReduce of two scalars.

Schedule notes:
- The first collective on a fresh execution costs ~60-80us of firmware boot;
  a dummy AllReduce at kernel start absorbs it concurrently with the
  projections. Everything before the real AllReduce result (~95us) is gate
  shadow; warm-keeper matmuls (in the AV psum pool, emitted after the
  pre-gate S matmuls) bridge the PE so the clock gate stays 8/8.
- PSUM: S-logit pool = 2 x [128,3,512] (6 banks); AV/O pool = 1 x [128,2,512]
  (2 banks). S tiles hold 3 (skt, head) chunks -> one exp call each (1536
  elems; ragged 1-chunk tail), and the AV matmuls + O-projection chunks live
  in their own 2-bank ring so they never stall the S/exp pipeline.
- E layout interleaves the two heads (chunk 2*skc + i) so S matmul pairs are
  adjacent 64-row tiles on row groups (0,0)/(64,0) -> concurrent.
- Q/K/S run fp32r (exp amplifies absolute logit error; bf16 there costs ~2%
  output error). V is computed fp32r but stored bf16; AV and O run bf16.
- AV output (z + ones-row denominators) is copied to SBUF immediately so the
  psum slot recycles without waiting for the normalize chain.
- dma_start costs ~0.6us serial issue on the Sync engine -> few, large DMAs.
- The RMS scalar uses exp(0.5*ln(m)) + one Newton step; a dummy Ln up front
  makes walrus load the natural_log_exp table set once for the whole kernel.
"""

import os as _os
import sys

sys.path.insert(0, "/opt/trn_rl_repo")

import numpy as np
from ml_dtypes import bfloat16

import concourse.bacc as bacc
import concourse.mybir as mybir
from concourse import library_config, tile
from concourse.bass_utils import run_bass_kernel_spmd

F32 = mybir.dt.float32
F32R = mybir.dt.float32r
BF16 = mybir.dt.bfloat16
AF = mybir.ActivationFunctionType
ALU = mybir.AluOpType
AX = mybir.AxisListType

P = 128
D = 1024
S = 1024
N_HEAD = 16
DH = 64
NHL = 8          # heads per core
DC = 8           # d contraction chunks of 128
EPS = 1e-6
COUNT = 4 * 1024 * 1024   # elements of the full q (or k) tensor
N_KEEP = int(_os.environ.get("KN_KEEP", "55"))
USE_DUMMY_AR = _os.environ.get("KN_DUMMY", "1") == "1"
N_CORES = int(_os.environ.get("KN_CORES", "8"))
REPLICAS = [list(range(N_CORES))]

_CACHE = {}


def _rne11(x: np.ndarray) -> np.ndarray:
    """Round float32 to 11 explicit mantissa bits (matches HW float32r)."""
    u = np.ascontiguousarray(x, dtype=np.float32).view(np.uint32).astype(np.uint64)
    shift = 12
    bias = ((u >> shift) & 1) + ((1 << (shift - 1)) - 1)
    return (((u + bias) >> shift) << shift).astype(np.uint32).view(np.float32)


def _build():
    nc = bacc.Bacc("TRN2", target_bir_lowering=False, debug=False, num_devices=N_CORES)

    xt = nc.dram_tensor("xt", [P, DC, S], F32R, kind="ExternalInput")
    wqk = nc.dram_tensor("wqk", [P, 8, DC, P], F32R, kind="ExternalInput")
    wv = nc.dram_tensor("wv", [P, DC, NHL * DH], F32R, kind="ExternalInput")
    wo = nc.dram_tensor("wo", [P, 4, D], BF16, kind="ExternalInput")
    qscale = nc.dram_tensor("qscale", [P, 4], F32, kind="ExternalInput")
    zparta = nc.dram_tensor("zparta", [S, D], F32, kind="ExternalOutput")
    zpartb = nc.dram_tensor("zpartb", [S, D], F32, kind="ExternalOutput")

    with tile.TileContext(nc) as tc:
        with (
            tc.tile_pool(name="big", bufs=1) as big,
            tc.tile_pool(name="ep", bufs=2) as ep,
            tc.tile_pool(name="zp", bufs=1) as zp,
            tc.tile_pool(name="scr", bufs=2) as scrp,
            tc.tile_pool(name="ob", bufs=2) as obp,
            tc.tile_pool(name="aux", bufs=1) as auxp,
            tc.tile_pool(name="small", bufs=2) as smallp,
            tc.tile_pool(name="stats", bufs=1) as stp,
            tc.tile_pool(name="s3", bufs=2, space="PSUM") as s3p,
            tc.tile_pool(name="av", bufs=1, space="PSUM") as avp,
            tc.tile_pool(name="dram", bufs=1, space="DRAM") as dramp,
        ):
            # ---- persistent SBUF tensors ----
            xT = big.tile([P, DC, S], F32R, tag="xT")
            wqs = big.tile([P, 8, DC, P], F32R, tag="wqs")
            QT = big.tile([P, 4, S], F32R, tag="QT")
            KT = big.tile([P, 4, S], F32R, tag="KT")
            Vt = big.tile([P, 8, NHL, DH + 1], BF16, tag="Vt")
            zG = big.tile([P, 4, S], BF16, tag="zG")
            Wo_sb = big.tile([P, 4, D], BF16, tag="Wo")
            Wv_sb = big.tile([P, DC, NHL * DH], F32R, tag="Wv")

            qs_sb = stp.tile([P, 4], F32, tag="qs")
            sq_acc = stp.tile([P, 8], F32, tag="sqacc")
            qk2 = stp.tile([P, 2], F32, tag="qk2")
            g_sb = stp.tile([2, 1], F32, tag="gsb")
            gsum = stp.tile([1, 2], F32, tag="gsum")
            sc_a = stp.tile([1, 2], F32, tag="sca")
            sc_b = stp.tile([1, 2], F32, tag="scb")
            sc_c = stp.tile([1, 2], F32, tag="scc")
            pm = stp.tile([1, 1], F32, tag="pm")
            cinv = stp.tile([1, 1], F32, tag="cinv")
            c_bc = stp.tile([P, 1], F32, tag="cbc")
            dln = stp.tile([1, 1], F32, tag="dln")
            ones_col = stp.tile([P, 1], F32, tag="ones_col")
            ones_blk = stp.tile([P, 8, NHL, 1], F32, tag="ones_blk")

            nc.gpsimd.load_library(library_config.attn)
            if USE_DUMMY_AR:
                cc_warm_in = dramp.tile([2, 1], F32, tag="ccwi")
                cc_warm_out = dramp.tile([2, 1], F32, tag="ccwo",
                                         addr_space="Shared")
                nc.gpsimd.collective_compute(
                    "AllReduce",
                    ALU.add,
                    replica_groups=REPLICAS,
                    ins=[cc_warm_in[:]],
                    outs=[cc_warm_out[:]],
                )

            # ---- input DMAs (x and per-ct weights interleaved) ----
            nc.sync.dma_start(wqs[:, 0, :, :], wqk[:, 0, :, :])
            for dc in range(DC):
                nc.sync.dma_start(xT[:, dc, :], xt[:, dc, :])
                if dc >= 1:
                    nc.sync.dma_start(wqs[:, dc, :, :], wqk[:, dc, :, :])
            nc.sync.dma_start(qs_sb[:], qscale[:])
            nc.vector.memset(ones_col[:], 1.0)
            nc.vector.memset(ones_blk[:], 1.0)
            nc.vector.tensor_copy(Vt[:, :, :, DH : DH + 1], ones_blk[:])
            # preload the natural_log_exp ACT table set (covers Square, Ln,
            # Exp for the whole kernel -> no mid-kernel table switches)
            nc.scalar.activation(dln[:], pm[:], AF.Ln)
            nc.sync.dma_start(Wv_sb[:], wv[:])
            nc.sync.dma_start(Wo_sb[:], wo[:])

            # ---- phase A: q,k projections (ct-outer; x streams under dc) ----
            for ct in range(8):
                tA = s3p.tile([P, 3, 512], F32, tag="s3", name=f"tA{ct}")
                for dc in range(DC):
                    for st in range(2):
                        nc.tensor.matmul(
                            tA[:, st, :],
                            lhsT=wqs[:, ct, dc, :],
                            rhs=xT[:, dc, st * 512 : (st + 1) * 512],
                            start=(dc == 0),
                            stop=(dc == DC - 1),
                        )
                view = tA[:, 0:2, :]
                scr = scrp.tile([P, 2, 512], F32, tag="scr", name=f"sq{ct}")
                nc.scalar.activation(
                    scr[:], view, AF.Square, accum_out=sq_acc[:, ct : ct + 1]
                )
                if ct < 4:
                    nc.vector.tensor_scalar(
                        QT[:, ct, :],
                        view.rearrange("p a b -> p (a b)"),
                        qs_sb[:, ct : ct + 1],
                        None,
                        ALU.mult,
                    )
                else:
                    nc.vector.tensor_copy(
                        KT[:, ct - 4, :], view.rearrange("p a b -> p (a b)")
                    )

            # ---- global RMS: local reduce -> AllReduce ----
            nc.vector.reduce_sum(qk2[:, 0:1], sq_acc[:, 0:4], axis=AX.X)
            nc.vector.reduce_sum(qk2[:, 1:2], sq_acc[:, 4:8], axis=AX.X)
            g_ps = avp.tile([P, 2, 512], F32, tag="av", name="g_ps")
            nc.tensor.matmul(
                g_ps[0:2, 0, 0:1], lhsT=qk2[:], rhs=ones_col[:], start=True, stop=True
            )
            nc.vector.tensor_copy(g_sb[:], g_ps[0:2, 0, 0:1])
            cc_in = dramp.tile([2, 1], F32, tag="ccin")
            cc_out = dramp.tile([2, 1], F32, tag="ccout", addr_space="Shared")
            nc.sync.dma_start(cc_in[:], g_sb[:])
            nc.gpsimd.collective_compute(
                "AllReduce",
                ALU.add,
                replica_groups=REPLICAS,
                ins=[cc_in[:]],
                outs=[cc_out[:]],
            )
            nc.sync.dma_start(gsum[:], cc_out[:].rearrange("a b -> b a"))

            # ---- V projection (fp32r, stored bf16), in the gate shadow ----
            for k in range(3):
                sms = range(3 * k, min(3 * k + 3, 8))
                tV = s3p.tile([P, 3, 512], F32, tag="s3", name=f"tV{k}")
                for j, sm in enumerate(sms):
                    for dc in range(DC):
                        nc.tensor.matmul(
                            tV[:, j, :],
                            lhsT=xT[:, dc, sm * P : (sm + 1) * P],
                            rhs=Wv_sb[:, dc, :],
                            start=(dc == 0),
                            stop=(dc == DC - 1),
                        )
                n = len(sms)
                nc.vector.tensor_copy(
                    Vt[:, 3 * k : 3 * k + n, :, 0:DH],
                    tV[:, 0:n, :].rearrange("p a (h d) -> p a h d", h=NHL),
                )

            # ---- RMS scalar chain: sqrt(m) = exp(0.5 ln m), one Newton step ----
            nc.vector.tensor_scalar_mul(sc_a[:], gsum[:], 1.0 / COUNT)   # m
            nc.scalar.activation(sc_c[:], sc_a[:], AF.Ln)                # ln m
            nc.scalar.activation(sc_b[:], sc_c[:], AF.Exp, scale=0.5)    # r0
            nc.vector.reciprocal(sc_c[:], sc_b[:])                       # 1/r0
            nc.vector.tensor_mul(sc_c[:], sc_a[:], sc_c[:])              # m/r0
            nc.vector.tensor_add(sc_b[:], sc_b[:], sc_c[:])              # r0 + m/r0
            nc.vector.tensor_scalar(sc_b[:], sc_b[:], 0.5, EPS, ALU.mult, ALU.add)
            nc.vector.tensor_mul(pm[:], sc_b[:, 0:1], sc_b[:, 1:2])
            nc.vector.reciprocal(cinv[:], pm[:])
            nc.gpsimd.partition_broadcast(c_bc[:], cinv[:])

            def o_chunk(sm, gg0, out_dram):
                tO = avp.tile([P, 2, 512], F32, tag="av", name=f"tO_{gg0}_{sm}")
                for nt in range(2):
                    for gg in (gg0, gg0 + 1):
                        nc.tensor.matmul(
                            tO[:, nt, :],
                            lhsT=zG[:, gg, sm * P : (sm + 1) * P],
                            rhs=Wo_sb[:, gg, nt * 512 : (nt + 1) * 512],
                            start=(gg == gg0),
                            stop=(gg == gg0 + 1),
                        )
                ob = obp.tile([P, 2, 512], F32, tag="ob", name=f"ob_{gg0}_{sm}")
                nc.vector.tensor_copy(ob[:], tO[:])
                nc.sync.dma_start(
                    out_dram[sm * P : (sm + 1) * P, :],
                    ob[:].rearrange("p a b -> p (a b)"),
                )

            def wave_fillers(g, t, E_mix, o_specs):
                """Emission closures finishing wave (g,t): AV quartets, the
                normalize chain, and O-projection chunks. Interleaved between
                the NEXT wave's S tiles so the PE stays uniformly busy."""
                tsl = slice(t * 512, (t + 1) * 512)
                state = {}

                def av_quartet(i, lo):
                    def emit():
                        if (i, lo) == (0, 0):
                            state["tAV"] = avp.tile([P, 2, 512], F32, tag="av",
                                                    name=f"tAV_{g}_{t}")
                        tAV = state["tAV"]
                        l = 2 * g + i
                        for skc in range(lo, lo + 4):
                            nc.tensor.matmul(
                                tAV[0 : DH + 1, i, :],
                                lhsT=Vt[:, skc, l, :],
                                rhs=E_mix[:, 2 * skc + i, :],
                                start=(skc == 0),
                                stop=(skc == 7),
                            )
                    return emit

                def av_done():
                    # raw z + denominators to SBUF so the psum slot recycles
                    tAV = state["tAV"]
                    zAV = zp.tile([DH, 2, 512], F32, tag="zav", name=f"zAV_{g}_{t}")
                    zden = auxp.tile([1, 2, 512], F32, tag="zden",
                                     name=f"zd_{g}_{t}")
                    nc.vector.tensor_copy(zAV[:], tAV[0:DH, 0:2, :])
                    nc.vector.tensor_copy(zden[:], tAV[DH : DH + 1, 0:2, :])
                    rs_r = auxp.tile([1, 2, 512], F32, tag="rs", name=f"rr_{g}_{t}")
                    nc.vector.reciprocal_approx_fast(rs_r[:], zden[:])
                    for i in range(2):
                        bc_sb = smallp.tile([DH, 512], F32, tag="bcs",
                                            name=f"bc_{g}_{t}_{i}")
                        nc.gpsimd.partition_broadcast(bc_sb[:], rs_r[:, i, :])
                        if i == 0:
                            nc.vector.tensor_mul(
                                zG[0:DH, g, tsl], zAV[:, 0, :], bc_sb[:]
                            )
                        else:
                            ztmp = auxp.tile([DH, 512], BF16, tag="ztmp",
                                             name=f"zt_{g}_{t}")
                            nc.vector.tensor_mul(ztmp[:], zAV[:, 1, :], bc_sb[:])
                            nc.sync.dma_start(zG[DH:P, g, tsl], ztmp[:])

                def o_emit(spec):
                    def emit():
                        gg0, sm, dram = spec
                        o_chunk(sm, gg0, dram)
                    return emit

                fillers = [
                    av_quartet(0, 0),
                    av_quartet(0, 4),
                    av_quartet(1, 0),
                    lambda: (av_quartet(1, 4)(), av_done()),
                ]
                for spec in o_specs:
                    fillers.append(o_emit(spec))
                while len(fillers) < 6:
                    fillers.append(lambda: None)
                return fillers

            def s_tile(g, t, k):
                """S-logit psum tile k (chunks 3k..3k+2) and its exp."""
                tsl = slice(t * 512, (t + 1) * 512)
                cs = range(3 * k, min(3 * k + 3, 16))
                tS = s3p.tile([P, 3, 512], F32, tag="s3", name=f"tS_{g}_{t}_{k}")
                for c in cs:
                    skt, i = c // 2, c % 2
                    hp = i * DH
                    nc.tensor.matmul(
                        tS[:, c - 3 * k, :],
                        lhsT=KT[hp : hp + DH, g, skt * P : (skt + 1) * P],
                        rhs=QT[hp : hp + DH, g, tsl],
                        start=True,
                        stop=True,
                    )
                return tS, cs

            def s_exp(E_mix, tS, cs, k):
                n = len(cs)
                nc.scalar.activation(
                    E_mix[:, 3 * k : 3 * k + n, :], tS[:, 0:n, :], AF.Exp,
                    scale=c_bc[:, 0:1]
                )

            # ---- attention: per (head-pair g, q-half t) wave ----
            # 16 S chunks per wave, chunk c = 2*skt + i; psum tile k = c//3;
            # one exp per tile into E_mix[:, 3k:3k+3, :]. Wave w's AV matmuls,
            # normalize chain, and O chunks are emitted as fillers between
            # wave w+1's S tiles so the PE stays uniformly busy and ACT runs
            # exps back to back. t-outer wave order lets 12 of 16 O chunks
            # run inline; only the (head pairs 2,3) x (sm 4-7) chunks remain
            # for the tail.
            waves = [(g, t) for t in range(2) for g in range(4)]
            # O-chunk queue: (gg0, sm, out_dram), ready once the needed waves'
            # z is finalized (tracked implicitly by emission position).
            o_queue = (
                [(0, sm, zparta) for sm in range(4)]
                + [(2, sm, zpartb) for sm in range(4)]
                + [(0, sm, zparta) for sm in range(4, 8)]
                + [(2, sm, zpartb) for sm in range(4, 8)]
            )
            o_pos = 0
            fillers = None
            for w, (g, t) in enumerate(waves):
                E_mix = ep.tile([P, 16, 512], BF16, tag="E", name=f"E_{g}_{t}")
                for k in range(6):
                    tS, cs = s_tile(g, t, k)
                    if w == 0 and k == 1:
                        # Warm keepers bridge the PE to the AllReduce result
                        # (the exps below gate on c_bc). They live in the AV
                        # pool so they never block the S/exp ring.
                        wk = avp.tile([P, 2, 512], F32, tag="av", name="wk")
                        for r in range(N_KEEP):
                            nc.tensor.matmul(
                                wk[:, r % 2, :],
                                lhsT=KT[:, 0, 0:P],
                                rhs=KT[:, 0, 0:512],
                                start=True,
                                stop=True,
                            )
                    if fillers is not None:
                        fillers[k]()
                    s_exp(E_mix, tS, cs, k)
                n_o = 2 if w >= 2 else 0
                specs = o_queue[o_pos : o_pos + n_o]
                o_pos += n_o
                fillers = wave_fillers(g, t, E_mix, specs)
            # tail: finish the last wave, then the remaining O chunks through
            # the now-free 2-slot S ring
            for f in fillers:
                f()
            for gg0, sm, dram in o_queue[o_pos:]:
                tO = s3p.tile([P, 3, 512], F32, tag="s3", name=f"tO2_{sm}")
                for nt in range(2):
                    for gg in (gg0, gg0 + 1):
                        nc.tensor.matmul(
                            tO[:, nt, :],
                            lhsT=zG[:, gg, sm * P : (sm + 1) * P],
                            rhs=Wo_sb[:, gg, nt * 512 : (nt + 1) * 512],
                            start=(gg == gg0),
                            stop=(gg == gg0 + 1),
                        )
                ob = obp.tile([P, 2, 512], F32, tag="ob", name=f"ob2_{gg0}_{sm}")
                nc.vector.tensor_copy(ob[:], tO[:, 0:2, :])
                nc.sync.dma_start(
                    dram[sm * P : (sm + 1) * P, :],
                    ob[:].rearrange("p a b -> p (a b)"),
                )

    nc.compile()
    return nc


def _get_nc():
    if "nc" not in _CACHE:
        _CACHE["nc"] = _build()
    return _CACHE["nc"]


def _prep_core_inputs(x, Wqkv, Wo, scale_q, scale_k):
    """Host-side shard + layout prep. Returns list of 8 in_maps."""
    x = np.asarray(x, dtype=np.float32)
    Wqkv = np.asarray(Wqkv, dtype=np.float32)
    Wo = np.asarray(Wo, dtype=np.float32)
    scale_q = np.asarray(scale_q, dtype=np.float32)
    scale_k = np.asarray(scale_k, dtype=np.float32)

    # combined per-d_head scale folded into Q (applied after raw sum-sq)
    qs_vec = np.tile(scale_q * scale_k, NHL)               # [512]
    qs_dev = np.ascontiguousarray(qs_vec.reshape(4, P).T)  # [128,4]

    xt_all = []
    for b in range(4):
        xTb = x[b].T                                       # [d, s]
        lay = xTb.reshape(DC, P, S).transpose(1, 0, 2)     # [128, 8, 1024]
        xt_all.append(np.ascontiguousarray(_rne11(lay)))

    in_maps = []
    for c in range(8):
        b = c // 2
        hh = (c % 2) * NHL
        cols = slice(hh * DH, (hh + NHL) * DH)
        wq_c = Wqkv[:, 0 * D:1 * D][:, cols]               # [1024, 512]
        wk_c = Wqkv[:, 1 * D:2 * D][:, cols]
        wv_c = Wqkv[:, 2 * D:3 * D][:, cols]
        wqk_c = _rne11(np.concatenate([wq_c, wk_c], axis=1))  # [1024, 1024]
        # [p, ct, dc, n]: one DMA per ct covers the full contraction
        wqk_dev = np.ascontiguousarray(
            wqk_c.reshape(DC, P, 8, P).transpose(1, 2, 0, 3)
        )
        wv_dev = np.ascontiguousarray(
            _rne11(wv_c).reshape(DC, P, NHL * DH).transpose(1, 0, 2)
        )
        # Wo rows for local heads, arranged [128, 4, 1024]:
        # chunk g partition p = head (2g + p//64), row p%64
        wo_loc = Wo[(hh * DH):(hh + NHL) * DH, :]          # [512, 1024]
        wo_dev = np.empty((P, 4, D), dtype=bfloat16)
        for g in range(4):
            wo_dev[0:DH, g, :] = wo_loc[2 * g * DH:(2 * g + 1) * DH, :].astype(bfloat16)
            wo_dev[DH:P, g, :] = wo_loc[(2 * g + 1) * DH:(2 * g + 2) * DH, :].astype(bfloat16)
        in_maps.append(
            {
                "xt": xt_all[b],
                "wqk": wqk_dev,
                "wv": wv_dev,
                "wo": np.ascontiguousarray(wo_dev),
                "qscale": qs_dev,
            }
        )
    return in_maps


def run(x, Wqkv, Wo, scale_q, scale_k, trace=False):
    nc = _get_nc()
    in_maps = _prep_core_inputs(x, Wqkv, Wo, scale_q, scale_k)
    res = run_bass_kernel_spmd(
        nc, in_maps[:N_CORES], core_ids=list(range(N_CORES)), trace=trace
    )
    out = np.empty((4, S, D), dtype=np.float32)
    for b in range(4):
        if N_CORES == 8:
            out[b] = (
                res.results[2 * b]["zparta"]
                + res.results[2 * b]["zpartb"]
                + res.results[2 * b + 1]["zparta"]
                + res.results[2 * b + 1]["zpartb"]
            )
    return out, res


def kernel(x, Wqkv, Wo, scale_q, scale_k):
    out, _ = run(x, Wqkv, Wo, scale_q, scale_k, trace=False)
    return out


# revision 20
# speedup vs baseline: 1.0559x; 1.0345x over previous
"""TRN2 Bass kernel for nn_Attention_188978561266.

Reference computation (b=4, s=1024, d=1024, 16 heads x 64):
    qkv = x @ Wqkv ; split q,k,v
    q = q / (sqrt(mean(q^2 over ALL elements)) + eps) * scale_q   (global scalar RMS)
    k = k / (sqrt(mean(k^2 over ALL elements)) + eps) * scale_k
    attn = softmax(q @ k^T)  (no 1/sqrt(d_head), no mask)
    out = (attn @ v) @ Wo

Sharding: 8 cores = (batch b in 0..3) x (head-half in 0..1). Each core computes
qkv for its batch restricted to its 8 heads, full attention for those heads,
and a partial output projection in two passes (zparta = head pairs 0,1 of the
local half, zpartb = head pairs 2,3). Host sums the four partials per batch.
The global RMS needs a cross-core AllReduce of two scalars.

Schedule notes:
- The first collective on a fresh execution costs ~60-80us of firmware boot;
  a dummy AllReduce at kernel start absorbs it concurrently with the
  projections. Everything before the real AllReduce result (~95us) is gate
  shadow; warm-keeper matmuls (in the AV psum pool, emitted after the
  pre-gate S matmuls) bridge the PE so the clock gate stays 8/8.
- PSUM: S-logit pool = 2 x [128,3,512] (6 banks); AV/O pool = 1 x [128,2,512]
  (2 banks). S tiles hold 3 (skt, head) chunks -> one exp call each (1536
  elems; ragged 1-chunk tail), and the AV matmuls + O-projection chunks live
  in their own 2-bank ring so they never stall the S/exp pipeline.
- E layout interleaves the two heads (chunk 2*skc + i) so S matmul pairs are
  adjacent 64-row tiles on row groups (0,0)/(64,0) -> concurrent.
- Q/K/S run fp32r (exp amplifies absolute logit error; bf16 there costs ~2%
  output error). V is computed fp32r but stored bf16; AV and O run bf16.
- AV output (z + ones-row denominators) is copied to SBUF immediately so the
  psum slot recycles without waiting for the normalize chain.
- dma_start costs ~0.6us serial issue on the Sync engine -> few, large DMAs.
- The RMS scalar uses exp(0.5*ln(m)) + one Newton step; a dummy Ln up front
  makes walrus load the natural_log_exp table set once for the whole kernel.
"""

import os as _os
import sys

sys.path.insert(0, "/opt/trn_rl_repo")

import numpy as np
from ml_dtypes import bfloat16

import concourse.bacc as bacc
import concourse.mybir as mybir
from concourse import library_config, tile
from concourse.bass_utils import run_bass_kernel_spmd

F32 = mybir.dt.float32
F32R = mybir.dt.float32r
BF16 = mybir.dt.bfloat16
AF = mybir.ActivationFunctionType
ALU = mybir.AluOpType
AX = mybir.AxisListType

P = 128
D = 1024
S = 1024
N_HEAD = 16
DH = 64
NHL = 8          # heads per core
DC = 8           # d contraction chunks of 128
EPS = 1e-6
COUNT = 4 * 1024 * 1024   # elements of the full q (or k) tensor
N_KEEP = int(_os.environ.get("KN_KEEP", "30"))
USE_DUMMY_AR = _os.environ.get("KN_DUMMY", "1") == "1"
N_CORES = int(_os.environ.get("KN_CORES", "8"))
REPLICAS = [list(range(N_CORES))]

_CACHE = {}


def _rne11(x: np.ndarray) -> np.ndarray:
    """Round float32 to 11 explicit mantissa bits (matches HW float32r)."""
    u = np.ascontiguousarray(x, dtype=np.float32).view(np.uint32).astype(np.uint64)
    shift = 12
    bias = ((u >> shift) & 1) + ((1 << (shift - 1)) - 1)
    return (((u + bias) >> shift) << shift).astype(np.uint32).view(np.float32)


def _build():
    nc = bacc.Bacc("TRN2", target_bir_lowering=False, debug=False, num_devices=N_CORES)

    xt = nc.dram_tensor("xt", [P, DC, S], F32R, kind="ExternalInput")
    wqk = nc.dram_tensor("wqk", [P, 8, DC, P], F32R, kind="ExternalInput")
    wv = nc.dram_tensor("wv", [P, DC, NHL * DH], F32R, kind="ExternalInput")
    wo = nc.dram_tensor("wo", [P, 4, D], BF16, kind="ExternalInput")
    qscale = nc.dram_tensor("qscale", [P, 4], F32, kind="ExternalInput")
    zparta = nc.dram_tensor("zparta", [S, D], F32, kind="ExternalOutput")
    zpartb = nc.dram_tensor("zpartb", [S, D], F32, kind="ExternalOutput")

    with tile.TileContext(nc) as tc:
        with (
            tc.tile_pool(name="big", bufs=1) as big,
            tc.tile_pool(name="ep", bufs=2) as ep,
            tc.tile_pool(name="zp", bufs=1) as zp,
            tc.tile_pool(name="scr", bufs=2) as scrp,
            tc.tile_pool(name="ob", bufs=2) as obp,
            tc.tile_pool(name="aux", bufs=1) as auxp,
            tc.tile_pool(name="small", bufs=2) as smallp,
            tc.tile_pool(name="stats", bufs=1) as stp,
            tc.tile_pool(name="s3", bufs=2, space="PSUM") as s3p,
            tc.tile_pool(name="av", bufs=1, space="PSUM") as avp,
            tc.tile_pool(name="dram", bufs=1, space="DRAM") as dramp,
        ):
            # ---- persistent SBUF tensors ----
            xT = big.tile([P, DC, S], F32R, tag="xT")
            wqs = big.tile([P, 8, DC, P], F32R, tag="wqs")
            QT = big.tile([P, 4, S], F32R, tag="QT")
            KT = big.tile([P, 4, S], F32R, tag="KT")
            Vt = big.tile([P, 8, NHL, DH + 1], BF16, tag="Vt")
            zG = big.tile([P, 4, S], BF16, tag="zG")
            Wo_sb = big.tile([P, 4, D], BF16, tag="Wo")
            Wv_sb = big.tile([P, DC, NHL * DH], F32R, tag="Wv")

            qs_sb = stp.tile([P, 4], F32, tag="qs")
            sq_acc = stp.tile([P, 8], F32, tag="sqacc")
            qk2 = stp.tile([P, 2], F32, tag="qk2")
            g_sb = stp.tile([2, 1], F32, tag="gsb")
            gsum = stp.tile([1, 2], F32, tag="gsum")
            sc_a = stp.tile([1, 2], F32, tag="sca")
            sc_b = stp.tile([1, 2], F32, tag="scb")
            sc_c = stp.tile([1, 2], F32, tag="scc")
            pm = stp.tile([1, 1], F32, tag="pm")
            cinv = stp.tile([1, 1], F32, tag="cinv")
            c_bc = stp.tile([P, 1], F32, tag="cbc")
            dln = stp.tile([1, 1], F32, tag="dln")
            ones_col = stp.tile([P, 1], F32, tag="ones_col")
            ones_blk = stp.tile([P, 8, NHL, 1], F32, tag="ones_blk")

            nc.gpsimd.load_library(library_config.attn)
            if USE_DUMMY_AR:
                cc_warm_in = dramp.tile([2, 1], F32, tag="ccwi")
                cc_warm_out = dramp.tile([2, 1], F32, tag="ccwo",
                                         addr_space="Shared")
                nc.gpsimd.collective_compute(
                    "AllReduce",
                    ALU.add,
                    replica_groups=REPLICAS,
                    ins=[cc_warm_in[:]],
                    outs=[cc_warm_out[:]],
                )

            # ---- input DMAs (x and per-ct weights interleaved) ----
            nc.sync.dma_start(wqs[:, 0, :, :], wqk[:, 0, :, :])
            for dc in range(DC):
                nc.sync.dma_start(xT[:, dc, :], xt[:, dc, :])
                if dc >= 1:
                    nc.sync.dma_start(wqs[:, dc, :, :], wqk[:, dc, :, :])
            nc.sync.dma_start(qs_sb[:], qscale[:])
            nc.vector.memset(ones_col[:], 1.0)
            nc.vector.memset(ones_blk[:], 1.0)
            nc.vector.tensor_copy(Vt[:, :, :, DH : DH + 1], ones_blk[:])
            # preload the natural_log_exp ACT table set (covers Square, Ln,
            # Exp for the whole kernel -> no mid-kernel table switches)
            nc.scalar.activation(dln[:], pm[:], AF.Ln)
            nc.sync.dma_start(Wv_sb[:], wv[:])
            nc.sync.dma_start(Wo_sb[:], wo[:])

            # ---- phase A: q,k projections (ct-outer; x streams under dc) ----
            for ct in range(8):
                tA = s3p.tile([P, 3, 512], F32, tag="s3", name=f"tA{ct}")
                for dc in range(DC):
                    for st in range(2):
                        nc.tensor.matmul(
                            tA[:, st, :],
                            lhsT=wqs[:, ct, dc, :],
                            rhs=xT[:, dc, st * 512 : (st + 1) * 512],
                            start=(dc == 0),
                            stop=(dc == DC - 1),
                        )
                view = tA[:, 0:2, :]
                scr = scrp.tile([P, 2, 512], F32, tag="scr", name=f"sq{ct}")
                nc.scalar.activation(
                    scr[:], view, AF.Square, accum_out=sq_acc[:, ct : ct + 1]
                )
                if ct < 4:
                    nc.vector.tensor_scalar(
                        QT[:, ct, :],
                        view.rearrange("p a b -> p (a b)"),
                        qs_sb[:, ct : ct + 1],
                        None,
                        ALU.mult,
                    )
                else:
                    nc.vector.tensor_copy(
                        KT[:, ct - 4, :], view.rearrange("p a b -> p (a b)")
                    )

            # ---- global RMS: local reduce -> AllReduce ----
            nc.vector.reduce_sum(qk2[:, 0:1], sq_acc[:, 0:4], axis=AX.X)
            nc.vector.reduce_sum(qk2[:, 1:2], sq_acc[:, 4:8], axis=AX.X)
            g_ps = avp.tile([P, 2, 512], F32, tag="av", name="g_ps")
            nc.tensor.matmul(
                g_ps[0:2, 0, 0:1], lhsT=qk2[:], rhs=ones_col[:], start=True, stop=True
            )
            nc.vector.tensor_copy(g_sb[:], g_ps[0:2, 0, 0:1])
            cc_in = dramp.tile([2, 1], F32, tag="ccin")
            cc_out = dramp.tile([2, 1], F32, tag="ccout", addr_space="Shared")
            nc.sync.dma_start(cc_in[:], g_sb[:])
            nc.gpsimd.collective_compute(
                "AllReduce",
                ALU.add,
                replica_groups=REPLICAS,
                ins=[cc_in[:]],
                outs=[cc_out[:]],
            )
            nc.sync.dma_start(gsum[:], cc_out[:].rearrange("a b -> b a"))

            # ---- V projection (fp32r, stored bf16), in the gate shadow ----
            for k in range(3):
                sms = range(3 * k, min(3 * k + 3, 8))
                tV = s3p.tile([P, 3, 512], F32, tag="s3", name=f"tV{k}")
                for j, sm in enumerate(sms):
                    for dc in range(DC):
                        nc.tensor.matmul(
                            tV[:, j, :],
                            lhsT=xT[:, dc, sm * P : (sm + 1) * P],
                            rhs=Wv_sb[:, dc, :],
                            start=(dc == 0),
                            stop=(dc == DC - 1),
                        )
                n = len(sms)
                nc.vector.tensor_copy(
                    Vt[:, 3 * k : 3 * k + n, :, 0:DH],
                    tV[:, 0:n, :].rearrange("p a (h d) -> p a h d", h=NHL),
                )

            # ---- RMS scalar chain: sqrt(m) = exp(0.5 ln m), one Newton step ----
            nc.vector.tensor_scalar_mul(sc_a[:], gsum[:], 1.0 / COUNT)   # m
            nc.scalar.activation(sc_c[:], sc_a[:], AF.Ln)                # ln m
            nc.scalar.activation(sc_b[:], sc_c[:], AF.Exp, scale=0.5)    # r0
            nc.vector.reciprocal(sc_c[:], sc_b[:])                       # 1/r0
            nc.vector.tensor_mul(sc_c[:], sc_a[:], sc_c[:])              # m/r0
            nc.vector.tensor_add(sc_b[:], sc_b[:], sc_c[:])              # r0 + m/r0
            nc.vector.tensor_scalar(sc_b[:], sc_b[:], 0.5, EPS, ALU.mult, ALU.add)
            nc.vector.tensor_mul(pm[:], sc_b[:, 0:1], sc_b[:, 1:2])
            nc.vector.reciprocal(cinv[:], pm[:])
            nc.gpsimd.partition_broadcast(c_bc[:], cinv[:])

            def o_chunk(sm, gg0, out_dram):
                tO = avp.tile([P, 2, 512], F32, tag="av", name=f"tO_{gg0}_{sm}")
                for nt in range(2):
                    for gg in (gg0, gg0 + 1):
                        nc.tensor.matmul(
                            tO[:, nt, :],
                            lhsT=zG[:, gg, sm * P : (sm + 1) * P],
                            rhs=Wo_sb[:, gg, nt * 512 : (nt + 1) * 512],
                            start=(gg == gg0),
                            stop=(gg == gg0 + 1),
                        )
                ob = obp.tile([P, 2, 512], F32, tag="ob", name=f"ob_{gg0}_{sm}")
                nc.vector.tensor_copy(ob[:], tO[:])
                nc.sync.dma_start(
                    out_dram[sm * P : (sm + 1) * P, :],
                    ob[:].rearrange("p a b -> p (a b)"),
                )

            def wave_fillers(g, t, E_mix, o_specs):
                """Emission closures finishing wave (g,t): AV quartets, the
                normalize chain, and O-projection chunks. Interleaved between
                the NEXT wave's S tiles so the PE stays uniformly busy."""
                tsl = slice(t * 512, (t + 1) * 512)
                state = {}

                def av_quartet(i, lo):
                    def emit():
                        if (i, lo) == (0, 0):
                            state["tAV"] = avp.tile([P, 2, 512], F32, tag="av",
                                                    name=f"tAV_{g}_{t}")
                        tAV = state["tAV"]
                        l = 2 * g + i
                        for skc in range(lo, lo + 4):
                            nc.tensor.matmul(
                                tAV[0 : DH + 1, i, :],
                                lhsT=Vt[:, skc, l, :],
                                rhs=E_mix[:, 2 * skc + i, :],
                                start=(skc == 0),
                                stop=(skc == 7),
                            )
                    return emit

                def av_done():
                    # raw z + denominators to SBUF so the psum slot recycles
                    tAV = state["tAV"]
                    zAV = zp.tile([DH, 2, 512], F32, tag="zav", name=f"zAV_{g}_{t}")
                    zden = auxp.tile([1, 2, 512], F32, tag="zden",
                                     name=f"zd_{g}_{t}")
                    nc.vector.tensor_copy(zAV[:], tAV[0:DH, 0:2, :])
                    nc.vector.tensor_copy(zden[:], tAV[DH : DH + 1, 0:2, :])
                    rs_r = auxp.tile([1, 2, 512], F32, tag="rs", name=f"rr_{g}_{t}")
                    nc.vector.reciprocal_approx_fast(rs_r[:], zden[:])
                    for i in range(2):
                        bc_sb = smallp.tile([DH, 512], F32, tag="bcs",
                                            name=f"bc_{g}_{t}_{i}")
                        nc.gpsimd.partition_broadcast(bc_sb[:], rs_r[:, i, :])
                        if i == 0:
                            nc.vector.tensor_mul(
                                zG[0:DH, g, tsl], zAV[:, 0, :], bc_sb[:]
                            )
                        else:
                            ztmp = auxp.tile([DH, 512], BF16, tag="ztmp",
                                             name=f"zt_{g}_{t}")
                            nc.vector.tensor_mul(ztmp[:], zAV[:, 1, :], bc_sb[:])
                            nc.sync.dma_start(zG[DH:P, g, tsl], ztmp[:])

                def o_emit(spec):
                    def emit():
                        gg0, sm, dram = spec
                        o_chunk(sm, gg0, dram)
                    return emit

                o_fill = [o_emit(s) for s in o_specs]
                o_fill += [lambda: None] * (2 - len(o_fill))
                q34 = av_quartet(1, 4)
                fillers = [
                    av_quartet(0, 0),
                    av_quartet(0, 4),
                    o_fill[0],
                    av_quartet(1, 0),
                    lambda: (q34(), o_fill[1]()),
                    av_done,
                ]
                return fillers

            def s_tile(g, t, k):
                """S-logit psum tile k (chunks 3k..3k+2) and its exp."""
                tsl = slice(t * 512, (t + 1) * 512)
                cs = range(3 * k, min(3 * k + 3, 16))
                tS = s3p.tile([P, 3, 512], F32, tag="s3", name=f"tS_{g}_{t}_{k}")
                for c in cs:
                    skt, i = c // 2, c % 2
                    hp = i * DH
                    nc.tensor.matmul(
                        tS[:, c - 3 * k, :],
                        lhsT=KT[hp : hp + DH, g, skt * P : (skt + 1) * P],
                        rhs=QT[hp : hp + DH, g, tsl],
                        start=True,
                        stop=True,
                    )
                return tS, cs

            def s_exp(E_mix, tS, cs, k):
                n = len(cs)
                nc.scalar.activation(
                    E_mix[:, 3 * k : 3 * k + n, :], tS[:, 0:n, :], AF.Exp,
                    scale=c_bc[:, 0:1]
                )

            # ---- attention: per (head-pair g, q-half t) wave ----
            # 16 S chunks per wave, chunk c = 2*skt + i; psum tile k = c//3;
            # one exp per tile into E_mix[:, 3k:3k+3, :]. Wave w's AV matmuls,
            # normalize chain, and O chunks are emitted as fillers between
            # wave w+1's S tiles so the PE stays uniformly busy and ACT runs
            # exps back to back. t-outer wave order lets 12 of 16 O chunks
            # run inline; only the (head pairs 2,3) x (sm 4-7) chunks remain
            # for the tail.
            waves = [(g, t) for t in range(2) for g in range(4)]
            # O-chunk queue: (gg0, sm, out_dram), ready once the needed waves'
            # z is finalized (tracked implicitly by emission position).
            o_queue = (
                [(0, sm, zparta) for sm in range(4)]
                + [(2, sm, zpartb) for sm in range(4)]
                + [(0, sm, zparta) for sm in range(4, 8)]
                + [(2, sm, zpartb) for sm in range(4, 8)]
            )
            o_pos = 0
            fillers = None
            for w, (g, t) in enumerate(waves):
                E_mix = ep.tile([P, 16, 512], BF16, tag="E", name=f"E_{g}_{t}")
                for k in range(6):
                    tS, cs = s_tile(g, t, k)
                    if w == 0 and k == 1:
                        # Warm keepers bridge the PE to the AllReduce result
                        # (the exps below gate on c_bc). They live in the AV
                        # pool so they never block the S/exp ring.
                        wk = avp.tile([P, 2, 512], F32, tag="av", name="wk")
                        for r in range(N_KEEP):
                            nc.tensor.matmul(
                                wk[:, r % 2, :],
                                lhsT=KT[:, 0, 0:P],
                                rhs=KT[:, 0, 0:512],
                                start=True,
                                stop=True,
                            )
                    if fillers is not None:
                        fillers[k]()
                    s_exp(E_mix, tS, cs, k)
                n_o = 2 if 2 <= w < 7 else 0
                specs = o_queue[o_pos : o_pos + n_o]
                o_pos += n_o
                fillers = wave_fillers(g, t, E_mix, specs)
            # tail: finish the last wave, then the remaining O chunks through
            # the now-free 2-slot S ring
            for f in fillers:
                f()
            for gg0, sm, dram in o_queue[o_pos:]:
                tO = s3p.tile([P, 3, 512], F32, tag="s3", name=f"tO2_{sm}")
                for nt in range(2):
                    for gg in (gg0, gg0 + 1):
                        nc.tensor.matmul(
                            tO[:, nt, :],
                            lhsT=zG[:, gg, sm * P : (sm + 1) * P],
                            rhs=Wo_sb[:, gg, nt * 512 : (nt + 1) * 512],
                            start=(gg == gg0),
                            stop=(gg == gg0 + 1),
                        )
                ob = obp.tile([P, 2, 512], F32, tag="ob", name=f"ob2_{gg0}_{sm}")
                nc.vector.tensor_copy(ob[:], tO[:, 0:2, :])
                nc.sync.dma_start(
                    dram[sm * P : (sm + 1) * P, :],
                    ob[:].rearrange("p a b -> p (a b)"),
                )

    nc.compile()
    return nc


def _get_nc():
    if "nc" not in _CACHE:
        _CACHE["nc"] = _build()
    return _CACHE["nc"]


def _prep_core_inputs(x, Wqkv, Wo, scale_q, scale_k):
    """Host-side shard + layout prep. Returns list of 8 in_maps."""
    x = np.asarray(x, dtype=np.float32)
    Wqkv = np.asarray(Wqkv, dtype=np.float32)
    Wo = np.asarray(Wo, dtype=np.float32)
    scale_q = np.asarray(scale_q, dtype=np.float32)
    scale_k = np.asarray(scale_k, dtype=np.float32)

    # combined per-d_head scale folded into Q (applied after raw sum-sq)
    qs_vec = np.tile(scale_q * scale_k, NHL)               # [512]
    qs_dev = np.ascontiguousarray(qs_vec.reshape(4, P).T)  # [128,4]

    xt_all = []
    for b in range(4):
        xTb = x[b].T                                       # [d, s]
        lay = xTb.reshape(DC, P, S).transpose(1, 0, 2)     # [128, 8, 1024]
        xt_all.append(np.ascontiguousarray(_rne11(lay)))

    in_maps = []
    for c in range(8):
        b = c // 2
        hh = (c % 2) * NHL
        cols = slice(hh * DH, (hh + NHL) * DH)
        wq_c = Wqkv[:, 0 * D:1 * D][:, cols]               # [1024, 512]
        wk_c = Wqkv[:, 1 * D:2 * D][:, cols]
        wv_c = Wqkv[:, 2 * D:3 * D][:, cols]
        wqk_c = _rne11(np.concatenate([wq_c, wk_c], axis=1))  # [1024, 1024]
        # [p, ct, dc, n]: one DMA per ct covers the full contraction
        wqk_dev = np.ascontiguousarray(
            wqk_c.reshape(DC, P, 8, P).transpose(1, 2, 0, 3)
        )
        wv_dev = np.ascontiguousarray(
            _rne11(wv_c).reshape(DC, P, NHL * DH).transpose(1, 0, 2)
        )
        # Wo rows for local heads, arranged [128, 4, 1024]:
        # chunk g partition p = head (2g + p//64), row p%64
        wo_loc = Wo[(hh * DH):(hh + NHL) * DH, :]          # [512, 1024]
        wo_dev = np.empty((P, 4, D), dtype=bfloat16)
        for g in range(4):
            wo_dev[0:DH, g, :] = wo_loc[2 * g * DH:(2 * g + 1) * DH, :].astype(bfloat16)
            wo_dev[DH:P, g, :] = wo_loc[(2 * g + 1) * DH:(2 * g + 2) * DH, :].astype(bfloat16)
        in_maps.append(
            {
                "xt": xt_all[b],
                "wqk": wqk_dev,
                "wv": wv_dev,
                "wo": np.ascontiguousarray(wo_dev),
                "qscale": qs_dev,
            }
        )
    return in_maps


def run(x, Wqkv, Wo, scale_q, scale_k, trace=False):
    nc = _get_nc()
    in_maps = _prep_core_inputs(x, Wqkv, Wo, scale_q, scale_k)
    res = run_bass_kernel_spmd(
        nc, in_maps[:N_CORES], core_ids=list(range(N_CORES)), trace=trace
    )
    out = np.empty((4, S, D), dtype=np.float32)
    for b in range(4):
        if N_CORES == 8:
            out[b] = (
                res.results[2 * b]["zparta"]
                + res.results[2 * b]["zpartb"]
                + res.results[2 * b + 1]["zparta"]
                + res.results[2 * b + 1]["zpartb"]
            )
    return out, res


def kernel(x, Wqkv, Wo, scale_q, scale_k):
    out, _ = run(x, Wqkv, Wo, scale_q, scale_k, trace=False)
    return out


# revision 21
# speedup vs baseline: 1.1040x; 1.0455x over previous
"""TRN2 Bass kernel for nn_Attention_188978561266.

Reference computation (b=4, s=1024, d=1024, 16 heads x 64):
    qkv = x @ Wqkv ; split q,k,v
    q = q / (sqrt(mean(q^2 over ALL elements)) + eps) * scale_q   (global scalar RMS)
    k = k / (sqrt(mean(k^2 over ALL elements)) + eps) * scale_k
    attn = softmax(q @ k^T)  (no 1/sqrt(d_head), no mask)
    out = (attn @ v) @ Wo

Sharding: 8 cores = (batch b in 0..3) x (head-half in 0..1). Each core computes
qkv for its batch restricted to its 8 heads, full attention for those heads,
and a partial output projection in two passes (zparta = head pairs 0,1 of the
local half, zpartb = head pairs 2,3). Host sums the four partials per batch.
The global RMS needs a cross-core AllReduce of two scalars.

Schedule notes:
- The first collective on a fresh execution costs ~60-80us of firmware boot;
  a dummy AllReduce at kernel start absorbs it concurrently with the
  projections. Everything before the real AllReduce result (~95us) is gate
  shadow; warm-keeper matmuls (in the AV psum pool, emitted after the
  pre-gate S matmuls) bridge the PE so the clock gate stays 8/8.
- PSUM: S-logit pool = 2 x [128,3,512] (6 banks); AV/O pool = 1 x [128,2,512]
  (2 banks). S tiles hold 3 (skt, head) chunks -> one exp call each (1536
  elems; ragged 1-chunk tail), and the AV matmuls + O-projection chunks live
  in their own 2-bank ring so they never stall the S/exp pipeline.
- E layout interleaves the two heads (chunk 2*skc + i) so S matmul pairs are
  adjacent 64-row tiles on row groups (0,0)/(64,0) -> concurrent.
- Q/K/S run fp32r (exp amplifies absolute logit error; bf16 there costs ~2%
  output error). V is computed fp32r but stored bf16; AV and O run bf16.
- AV output (z + ones-row denominators) is copied to SBUF immediately so the
  psum slot recycles without waiting for the normalize chain.
- dma_start costs ~0.6us serial issue on the Sync engine -> few, large DMAs.
- The RMS scalar uses exp(0.5*ln(m)) + one Newton step; a dummy Ln up front
  makes walrus load the natural_log_exp table set once for the whole kernel.
"""

import os as _os
import sys

sys.path.insert(0, "/opt/trn_rl_repo")

import numpy as np
from ml_dtypes import bfloat16

import concourse.bacc as bacc
import concourse.mybir as mybir
from concourse import library_config, tile
from concourse.bass_utils import run_bass_kernel_spmd

F32 = mybir.dt.float32
F32R = mybir.dt.float32r
BF16 = mybir.dt.bfloat16
AF = mybir.ActivationFunctionType
ALU = mybir.AluOpType
AX = mybir.AxisListType

P = 128
D = 1024
S = 1024
N_HEAD = 16
DH = 64
NHL = 8          # heads per core
DC = 8           # d contraction chunks of 128
EPS = 1e-6
COUNT = 4 * 1024 * 1024   # elements of the full q (or k) tensor
N_KEEP = int(_os.environ.get("KN_KEEP", "65"))
USE_DUMMY_AR = _os.environ.get("KN_DUMMY", "1") == "1"
N_CORES = int(_os.environ.get("KN_CORES", "8"))
REPLICAS = [list(range(N_CORES))]

_CACHE = {}


def _rne11(x: np.ndarray) -> np.ndarray:
    """Round float32 to 11 explicit mantissa bits (matches HW float32r)."""
    u = np.ascontiguousarray(x, dtype=np.float32).view(np.uint32).astype(np.uint64)
    shift = 12
    bias = ((u >> shift) & 1) + ((1 << (shift - 1)) - 1)
    return (((u + bias) >> shift) << shift).astype(np.uint32).view(np.float32)


def _build():
    nc = bacc.Bacc("TRN2", target_bir_lowering=False, debug=False, num_devices=N_CORES)

    xt = nc.dram_tensor("xt", [P, DC, S], F32R, kind="ExternalInput")
    wqk = nc.dram_tensor("wqk", [P, 8, DC, P], F32R, kind="ExternalInput")
    wv = nc.dram_tensor("wv", [P, DC, NHL * DH], F32R, kind="ExternalInput")
    wo = nc.dram_tensor("wo", [P, 4, D], BF16, kind="ExternalInput")
    qscale = nc.dram_tensor("qscale", [P, 4], F32, kind="ExternalInput")
    zparta = nc.dram_tensor("zparta", [S, D], F32, kind="ExternalOutput")
    zpartb = nc.dram_tensor("zpartb", [S, D], F32, kind="ExternalOutput")

    with tile.TileContext(nc) as tc:
        with (
            tc.tile_pool(name="big", bufs=1) as big,
            tc.tile_pool(name="ep", bufs=2) as ep,
            tc.tile_pool(name="zp", bufs=1) as zp,
            tc.tile_pool(name="scr", bufs=2) as scrp,
            tc.tile_pool(name="ob", bufs=2) as obp,
            tc.tile_pool(name="aux", bufs=1) as auxp,
            tc.tile_pool(name="small", bufs=2) as smallp,
            tc.tile_pool(name="stats", bufs=1) as stp,
            tc.tile_pool(name="s3", bufs=2, space="PSUM") as s3p,
            tc.tile_pool(name="av", bufs=1, space="PSUM") as avp,
            tc.tile_pool(name="dram", bufs=1, space="DRAM") as dramp,
        ):
            # ---- persistent SBUF tensors ----
            xT = big.tile([P, DC, S], F32R, tag="xT")
            wqs = big.tile([P, 8, DC, P], F32R, tag="wqs")
            QT = big.tile([P, 4, S], F32R, tag="QT")
            KT = big.tile([P, 4, S], F32R, tag="KT")
            Vt = big.tile([P, 8, NHL, DH + 1], BF16, tag="Vt")
            zG = big.tile([P, 4, S], BF16, tag="zG")
            Wo_sb = big.tile([P, 4, D], BF16, tag="Wo")
            Wv_sb = big.tile([P, DC, NHL * DH], F32R, tag="Wv")

            qs_sb = stp.tile([P, 4], F32, tag="qs")
            sq_acc = stp.tile([P, 8], F32, tag="sqacc")
            qk2 = stp.tile([P, 2], F32, tag="qk2")
            g_sb = stp.tile([2, 1], F32, tag="gsb")
            gsum = stp.tile([1, 2], F32, tag="gsum")
            sc_a = stp.tile([1, 2], F32, tag="sca")
            sc_b = stp.tile([1, 2], F32, tag="scb")
            sc_c = stp.tile([1, 2], F32, tag="scc")
            pm = stp.tile([1, 1], F32, tag="pm")
            cinv = stp.tile([1, 1], F32, tag="cinv")
            c_bc = stp.tile([P, 1], F32, tag="cbc")
            dln = stp.tile([1, 1], F32, tag="dln")
            ones_col = stp.tile([P, 1], F32, tag="ones_col")
            ones_blk = stp.tile([P, 8, NHL, 1], F32, tag="ones_blk")

            nc.gpsimd.load_library(library_config.attn)
            if USE_DUMMY_AR:
                cc_warm_in = dramp.tile([2, 1], F32, tag="ccwi")
                cc_warm_out = dramp.tile([2, 1], F32, tag="ccwo",
                                         addr_space="Shared")
                nc.gpsimd.collective_compute(
                    "AllReduce",
                    ALU.add,
                    replica_groups=REPLICAS,
                    ins=[cc_warm_in[:]],
                    outs=[cc_warm_out[:]],
                )

            # ---- input DMAs (x and per-ct weights interleaved) ----
            nc.sync.dma_start(wqs[:, 0, :, :], wqk[:, 0, :, :])
            for dc in range(DC):
                nc.sync.dma_start(xT[:, dc, :], xt[:, dc, :])
                if dc >= 1:
                    nc.sync.dma_start(wqs[:, dc, :, :], wqk[:, dc, :, :])
            nc.sync.dma_start(qs_sb[:], qscale[:])
            nc.vector.memset(ones_col[:], 1.0)
            nc.vector.memset(ones_blk[:], 1.0)
            nc.vector.tensor_copy(Vt[:, :, :, DH : DH + 1], ones_blk[:])
            # preload the natural_log_exp ACT table set (covers Square, Ln,
            # Exp for the whole kernel -> no mid-kernel table switches)
            nc.scalar.activation(dln[:], pm[:], AF.Ln)
            nc.sync.dma_start(Wv_sb[:], wv[:])
            nc.sync.dma_start(Wo_sb[:], wo[:])

            # ---- phase A: q,k projections (ct-outer; x streams under dc) ----
            for ct in range(8):
                tA = s3p.tile([P, 3, 512], F32, tag="s3", name=f"tA{ct}")
                for dc in range(DC):
                    for st in range(2):
                        nc.tensor.matmul(
                            tA[:, st, :],
                            lhsT=wqs[:, ct, dc, :],
                            rhs=xT[:, dc, st * 512 : (st + 1) * 512],
                            start=(dc == 0),
                            stop=(dc == DC - 1),
                        )
                view = tA[:, 0:2, :]
                scr = scrp.tile([P, 2, 512], F32, tag="scr", name=f"sq{ct}")
                nc.scalar.activation(
                    scr[:], view, AF.Square, accum_out=sq_acc[:, ct : ct + 1]
                )
                if ct < 4:
                    nc.vector.tensor_scalar(
                        QT[:, ct, :],
                        view.rearrange("p a b -> p (a b)"),
                        qs_sb[:, ct : ct + 1],
                        None,
                        ALU.mult,
                    )
                else:
                    nc.vector.tensor_copy(
                        KT[:, ct - 4, :], view.rearrange("p a b -> p (a b)")
                    )

            # ---- global RMS: local reduce -> AllReduce ----
            nc.vector.reduce_sum(qk2[:, 0:1], sq_acc[:, 0:4], axis=AX.X)
            nc.vector.reduce_sum(qk2[:, 1:2], sq_acc[:, 4:8], axis=AX.X)
            g_ps = avp.tile([P, 2, 512], F32, tag="av", name="g_ps")
            nc.tensor.matmul(
                g_ps[0:2, 0, 0:1], lhsT=qk2[:], rhs=ones_col[:], start=True, stop=True
            )
            nc.vector.tensor_copy(g_sb[:], g_ps[0:2, 0, 0:1])
            cc_in = dramp.tile([2, 1], F32, tag="ccin")
            cc_out = dramp.tile([2, 1], F32, tag="ccout", addr_space="Shared")
            nc.sync.dma_start(cc_in[:], g_sb[:])
            nc.gpsimd.collective_compute(
                "AllReduce",
                ALU.add,
                replica_groups=REPLICAS,
                ins=[cc_in[:]],
                outs=[cc_out[:]],
            )
            nc.sync.dma_start(gsum[:], cc_out[:].rearrange("a b -> b a"))

            # ---- V projection (fp32r, stored bf16), in the gate shadow ----
            for k in range(3):
                sms = range(3 * k, min(3 * k + 3, 8))
                tV = s3p.tile([P, 3, 512], F32, tag="s3", name=f"tV{k}")
                for j, sm in enumerate(sms):
                    for dc in range(DC):
                        nc.tensor.matmul(
                            tV[:, j, :],
                            lhsT=xT[:, dc, sm * P : (sm + 1) * P],
                            rhs=Wv_sb[:, dc, :],
                            start=(dc == 0),
                            stop=(dc == DC - 1),
                        )
                n = len(sms)
                nc.vector.tensor_copy(
                    Vt[:, 3 * k : 3 * k + n, :, 0:DH],
                    tV[:, 0:n, :].rearrange("p a (h d) -> p a h d", h=NHL),
                )

            # ---- RMS scalar chain: sqrt(m) = exp(0.5 ln m), one Newton step ----
            nc.vector.tensor_scalar_mul(sc_a[:], gsum[:], 1.0 / COUNT)   # m
            nc.scalar.activation(sc_c[:], sc_a[:], AF.Ln)                # ln m
            nc.scalar.activation(sc_b[:], sc_c[:], AF.Exp, scale=0.5)    # r0
            nc.vector.reciprocal(sc_c[:], sc_b[:])                       # 1/r0
            nc.vector.tensor_mul(sc_c[:], sc_a[:], sc_c[:])              # m/r0
            nc.vector.tensor_add(sc_b[:], sc_b[:], sc_c[:])              # r0 + m/r0
            nc.vector.tensor_scalar(sc_b[:], sc_b[:], 0.5, EPS, ALU.mult, ALU.add)
            nc.vector.tensor_mul(pm[:], sc_b[:, 0:1], sc_b[:, 1:2])
            nc.vector.reciprocal(cinv[:], pm[:])
            nc.gpsimd.partition_broadcast(c_bc[:], cinv[:])

            def o_chunk(sm, gg0, out_dram):
                tO = avp.tile([P, 2, 512], F32, tag="av", name=f"tO_{gg0}_{sm}")
                for nt in range(2):
                    for gg in (gg0, gg0 + 1):
                        nc.tensor.matmul(
                            tO[:, nt, :],
                            lhsT=zG[:, gg, sm * P : (sm + 1) * P],
                            rhs=Wo_sb[:, gg, nt * 512 : (nt + 1) * 512],
                            start=(gg == gg0),
                            stop=(gg == gg0 + 1),
                        )
                ob = obp.tile([P, 2, 512], F32, tag="ob", name=f"ob_{gg0}_{sm}")
                nc.vector.tensor_copy(ob[:], tO[:])
                nc.sync.dma_start(
                    out_dram[sm * P : (sm + 1) * P, :],
                    ob[:].rearrange("p a b -> p (a b)"),
                )

            def wave_fillers(g, t, E_mix, o_specs):
                """Emission closures finishing wave (g,t): AV quartets, the
                normalize chain, and O-projection chunks. Interleaved between
                the NEXT wave's S tiles so the PE stays uniformly busy."""
                tsl = slice(t * 512, (t + 1) * 512)
                state = {}

                def av_quartet(i, lo):
                    def emit():
                        if (i, lo) == (0, 0):
                            state["tAV"] = avp.tile([P, 2, 512], F32, tag="av",
                                                    name=f"tAV_{g}_{t}")
                        tAV = state["tAV"]
                        l = 2 * g + i
                        for skc in range(lo, lo + 4):
                            nc.tensor.matmul(
                                tAV[0 : DH + 1, i, :],
                                lhsT=Vt[:, skc, l, :],
                                rhs=E_mix[:, 2 * skc + i, :],
                                start=(skc == 0),
                                stop=(skc == 7),
                            )
                    return emit

                def av_done():
                    # raw z + denominators to SBUF so the psum slot recycles
                    tAV = state["tAV"]
                    zAV = zp.tile([DH, 2, 512], F32, tag="zav", name=f"zAV_{g}_{t}")
                    zden = auxp.tile([1, 2, 512], F32, tag="zden",
                                     name=f"zd_{g}_{t}")
                    nc.vector.tensor_copy(zAV[:], tAV[0:DH, 0:2, :])
                    nc.vector.tensor_copy(zden[:], tAV[DH : DH + 1, 0:2, :])
                    rs_r = auxp.tile([1, 2, 512], F32, tag="rs", name=f"rr_{g}_{t}")
                    nc.vector.reciprocal_approx_fast(rs_r[:], zden[:])
                    for i in range(2):
                        bc_sb = smallp.tile([DH, 512], F32, tag="bcs",
                                            name=f"bc_{g}_{t}_{i}")
                        nc.gpsimd.partition_broadcast(bc_sb[:], rs_r[:, i, :])
                        if i == 0:
                            nc.vector.tensor_mul(
                                zG[0:DH, g, tsl], zAV[:, 0, :], bc_sb[:]
                            )
                        else:
                            ztmp = auxp.tile([DH, 512], BF16, tag="ztmp",
                                             name=f"zt_{g}_{t}")
                            nc.vector.tensor_mul(ztmp[:], zAV[:, 1, :], bc_sb[:])
                            nc.sync.dma_start(zG[DH:P, g, tsl], ztmp[:])

                def o_emit(spec):
                    def emit():
                        gg0, sm, dram = spec
                        o_chunk(sm, gg0, dram)
                    return emit

                o_fill = [o_emit(s) for s in o_specs]
                o_fill += [lambda: None] * (2 - len(o_fill))
                q34 = av_quartet(1, 4)
                fillers = [
                    av_quartet(0, 0),
                    av_quartet(0, 4),
                    o_fill[0],
                    av_quartet(1, 0),
                    lambda: (q34(), o_fill[1]()),
                    av_done,
                ]
                return fillers

            def s_tile(g, t, k):
                """S-logit psum tile k (chunks 3k..3k+2) and its exp."""
                tsl = slice(t * 512, (t + 1) * 512)
                cs = range(3 * k, min(3 * k + 3, 16))
                tS = s3p.tile([P, 3, 512], F32, tag="s3", name=f"tS_{g}_{t}_{k}")
                for c in cs:
                    skt, i = c // 2, c % 2
                    hp = i * DH
                    nc.tensor.matmul(
                        tS[:, c - 3 * k, :],
                        lhsT=KT[hp : hp + DH, g, skt * P : (skt + 1) * P],
                        rhs=QT[hp : hp + DH, g, tsl],
                        start=True,
                        stop=True,
                    )
                return tS, cs

            def s_exp(E_mix, tS, cs, k):
                n = len(cs)
                nc.scalar.activation(
                    E_mix[:, 3 * k : 3 * k + n, :], tS[:, 0:n, :], AF.Exp,
                    scale=c_bc[:, 0:1]
                )

            # ---- attention: per (head-pair g, q-half t) wave ----
            # 16 S chunks per wave, chunk c = 2*skt + i; psum tile k = c//3;
            # one exp per tile into E_mix[:, 3k:3k+3, :]. Wave w's AV matmuls,
            # normalize chain, and O chunks are emitted as fillers between
            # wave w+1's S tiles so the PE stays uniformly busy and ACT runs
            # exps back to back. t-outer wave order lets 12 of 16 O chunks
            # run inline; only the (head pairs 2,3) x (sm 4-7) chunks remain
            # for the tail.
            waves = [(g, t) for t in range(2) for g in range(4)]
            # O-chunk queue: (gg0, sm, out_dram), ready once the needed waves'
            # z is finalized (tracked implicitly by emission position).
            o_queue = (
                [(0, sm, zparta) for sm in range(4)]
                + [(2, sm, zpartb) for sm in range(4)]
                + [(0, sm, zparta) for sm in range(4, 8)]
                + [(2, sm, zpartb) for sm in range(4, 8)]
            )
            o_pos = 0
            fillers = None
            for w, (g, t) in enumerate(waves):
                E_mix = ep.tile([P, 16, 512], BF16, tag="E", name=f"E_{g}_{t}")
                for k in range(6):
                    tS, cs = s_tile(g, t, k)
                    if w == 0 and k == 1:
                        # Warm keepers bridge the PE to the AllReduce result
                        # (the exps below gate on c_bc). They live in the AV
                        # pool so they never block the S/exp ring.
                        wk = avp.tile([P, 2, 512], F32, tag="av", name="wk")
                        for r in range(N_KEEP):
                            nc.tensor.matmul(
                                wk[:, r % 2, :],
                                lhsT=KT[:, 0, 0:P],
                                rhs=KT[:, 0, 0:512],
                                start=True,
                                stop=True,
                            )
                    if fillers is not None:
                        fillers[k]()
                    s_exp(E_mix, tS, cs, k)
                n_o = 2 if 2 <= w < 7 else 0
                specs = o_queue[o_pos : o_pos + n_o]
                o_pos += n_o
                fillers = wave_fillers(g, t, E_mix, specs)
            # tail: finish the last wave, then the remaining O chunks through
            # the now-free 2-slot S ring
            for f in fillers:
                f()
            for gg0, sm, dram in o_queue[o_pos:]:
                tO = s3p.tile([P, 3, 512], F32, tag="s3", name=f"tO2_{sm}")
                for nt in range(2):
                    for gg in (gg0, gg0 + 1):
                        nc.tensor.matmul(
                            tO[:, nt, :],
                            lhsT=zG[:, gg, sm * P : (sm + 1) * P],
                            rhs=Wo_sb[:, gg, nt * 512 : (nt + 1) * 512],
                            start=(gg == gg0),
                            stop=(gg == gg0 + 1),
                        )
                ob = obp.tile([P, 2, 512], F32, tag="ob", name=f"ob2_{gg0}_{sm}")
                nc.vector.tensor_copy(ob[:], tO[:, 0:2, :])
                nc.sync.dma_start(
                    dram[sm * P : (sm + 1) * P, :],
                    ob[:].rearrange("p a b -> p (a b)"),
                )

    nc.compile()
    return nc


def _get_nc():
    if "nc" not in _CACHE:
        _CACHE["nc"] = _build()
    return _CACHE["nc"]


def _prep_core_inputs(x, Wqkv, Wo, scale_q, scale_k):
    """Host-side shard + layout prep. Returns list of 8 in_maps."""
    x = np.asarray(x, dtype=np.float32)
    Wqkv = np.asarray(Wqkv, dtype=np.float32)
    Wo = np.asarray(Wo, dtype=np.float32)
    scale_q = np.asarray(scale_q, dtype=np.float32)
    scale_k = np.asarray(scale_k, dtype=np.float32)

    # combined per-d_head scale folded into Q (applied after raw sum-sq)
    qs_vec = np.tile(scale_q * scale_k, NHL)               # [512]
    qs_dev = np.ascontiguousarray(qs_vec.reshape(4, P).T)  # [128,4]

    xt_all = []
    for b in range(4):
        xTb = x[b].T                                       # [d, s]
        lay = xTb.reshape(DC, P, S).transpose(1, 0, 2)     # [128, 8, 1024]
        xt_all.append(np.ascontiguousarray(_rne11(lay)))

    in_maps = []
    for c in range(8):
        b = c // 2
        hh = (c % 2) * NHL
        cols = slice(hh * DH, (hh + NHL) * DH)
        wq_c = Wqkv[:, 0 * D:1 * D][:, cols]               # [1024, 512]
        wk_c = Wqkv[:, 1 * D:2 * D][:, cols]
        wv_c = Wqkv[:, 2 * D:3 * D][:, cols]
        wqk_c = _rne11(np.concatenate([wq_c, wk_c], axis=1))  # [1024, 1024]
        # [p, ct, dc, n]: one DMA per ct covers the full contraction
        wqk_dev = np.ascontiguousarray(
            wqk_c.reshape(DC, P, 8, P).transpose(1, 2, 0, 3)
        )
        wv_dev = np.ascontiguousarray(
            _rne11(wv_c).reshape(DC, P, NHL * DH).transpose(1, 0, 2)
        )
        # Wo rows for local heads, arranged [128, 4, 1024]:
        # chunk g partition p = head (2g + p//64), row p%64
        wo_loc = Wo[(hh * DH):(hh + NHL) * DH, :]          # [512, 1024]
        wo_dev = np.empty((P, 4, D), dtype=bfloat16)
        for g in range(4):
            wo_dev[0:DH, g, :] = wo_loc[2 * g * DH:(2 * g + 1) * DH, :].astype(bfloat16)
            wo_dev[DH:P, g, :] = wo_loc[(2 * g + 1) * DH:(2 * g + 2) * DH, :].astype(bfloat16)
        in_maps.append(
            {
                "xt": xt_all[b],
                "wqk": wqk_dev,
                "wv": wv_dev,
                "wo": np.ascontiguousarray(wo_dev),
                "qscale": qs_dev,
            }
        )
    return in_maps


def run(x, Wqkv, Wo, scale_q, scale_k, trace=False):
    nc = _get_nc()
    in_maps = _prep_core_inputs(x, Wqkv, Wo, scale_q, scale_k)
    res = run_bass_kernel_spmd(
        nc, in_maps[:N_CORES], core_ids=list(range(N_CORES)), trace=trace
    )
    out = np.empty((4, S, D), dtype=np.float32)
    for b in range(4):
        if N_CORES == 8:
            out[b] = (
                res.results[2 * b]["zparta"]
                + res.results[2 * b]["zpartb"]
                + res.results[2 * b + 1]["zparta"]
                + res.results[2 * b + 1]["zpartb"]
            )
    return out, res


def kernel(x, Wqkv, Wo, scale_q, scale_k):
    out, _ = run(x, Wqkv, Wo, scale_q, scale_k, trace=False)
    return out
